# revision 2
# baseline (speedup 1.0000x reference)
"""DeRA attention (2D-rope attention) Trainium2 kernel, 8-core head-parallel
with on-device collectives to minimize host<->device traffic.

Host I/O per core (the dominant cost under this harness):
 - up:   xcs_s [1728, 384] bf16  = this core's 1/8 sequence-slice of x^T
         stacked with its slice of the rope cos/sin tables (1.33 MB)
         wq/wk/wv head-shard [128, 12*192] bf16 (0.59 MB each)
         wo column-shard    [128, 12*192] bf16 (0.59 MB)
         biases [96, 2] f32 (tiny)
 - down: o_s [192, 3072] fp16 = this core's 192-row slice of o^T (1.18 MB)

Device dataflow:
 1. AllGather the x^T/table slices -> full x^T + full rope tables per core.
 2. Per-core head-parallel compute (2 of 16 heads): q/k/v projections,
    2D-rope (baked cos/sin mul + partition-block swap), dense attention
    with exp-sum-trick softmax (ones row appended to V gives the
    denominator inside the PV accumulation).
 3. AllGather the 16 per-head attention outputs (bf16) -> each core
    computes its own 192 columns of the output projection exactly in
    fp32 PSUM and downloads [192, S] fp16.
Host sums nothing: it concatenates the 8 row-slices and adds bo.
"""

import sys

if "/opt/trn_rl_repo" not in sys.path:
    sys.path.insert(0, "/opt/trn_rl_repo")

import numpy as np
import ml_dtypes

BF16N = ml_dtypes.bfloat16

# Problem config (hardcoded per spec)
S = 3072
DIM = 1536
NH = 16
HD = 96
NCORES = 8
GH, GW = 48, 64
SS = S // NCORES         # 384 sequence positions per core
KC = DIM // 128          # 12 contraction chunks of 128
NQ = S // 512            # 6
NKC = S // 128           # 24 key chunks
QB = 1024                # q block (columns per attention round)
NQB = S // QB            # 3
XR = DIM + 2 * HD        # 1728 rows in the fused x+tables upload
SCALE = 1.0 / float(np.sqrt(HD))

# per-head column permutation: [id evens, id odds, rot evens, rot odds]
_PERM = (
    [2 * j for j in range(16)]
    + [2 * j + 1 for j in range(16)]
    + [2 * j for j in range(16, 48)]
    + [2 * j + 1 for j in range(16, 48)]
)

_STATE = {}


def _build_nc():
    import concourse.bass as bass  # noqa: F401
    import concourse.tile as tile
    from concourse import bacc, mybir
    from concourse.masks import make_identity

    BF16 = mybir.dt.bfloat16
    F16 = mybir.dt.float16
    F32 = mybir.dt.float32
    AF = mybir.ActivationFunctionType
    OP = mybir.AluOpType

    nc = bacc.Bacc("TRN2", target_bir_lowering=False, debug=False,
                   num_devices=NCORES)

    xcsd = nc.dram_tensor("xcs_s", [XR, SS], BF16, kind="ExternalInput")
    wqd = nc.dram_tensor("wq_t", [128, KC * 192], BF16, kind="ExternalInput")
    wkd = nc.dram_tensor("wk_t", [128, KC * 192], BF16, kind="ExternalInput")
    wvd = nc.dram_tensor("wv_t", [128, KC * 192], BF16, kind="ExternalInput")
    wod = nc.dram_tensor("wo_t", [128, KC * 192], BF16, kind="ExternalInput")
    bqd = nc.dram_tensor("bq_t", [96, 2], F32, kind="ExternalInput")
    bkd = nc.dram_tensor("bk_t", [96, 2], F32, kind="ExternalInput")
    bvd = nc.dram_tensor("bv_t", [96, 2], F32, kind="ExternalInput")
    outd = nc.dram_tensor("o_s", [2 * HD, S], F16, kind="ExternalOutput")

    groups = [list(range(NCORES))]

    with tile.TileContext(nc) as tc:
        with tc.tile_pool(name="dram", bufs=1, space="DRAM") as dram:
            # ---- collective 1: gather x^T + rope tables ----
            xg_in = dram.tile([XR, SS], BF16, name="xg_in")
            nc.sync.dma_start(out=xg_in[:], in_=xcsd.ap())
            xg_out = dram.tile([NCORES, XR, SS], BF16, name="xg_out",
                               addr_space="Shared")
            nc.gpsimd.collective_compute(
                "AllGather", mybir.AluOpType.bypass, replica_groups=groups,
                ins=[xg_in[:]], outs=[xg_out[:]])
            ag_in = dram.tile([2 * HD, S], BF16, name="ag_in")
            ag_out = dram.tile([NCORES, 2 * HD, S], BF16, name="ag_out",
                               addr_space="Shared")

            const = tc.alloc_tile_pool(name="const", bufs=1)
            cos_sb = const.tile([96, S], BF16, name="cos_sb")
            sin_sb = const.tile([96, S], BF16, name="sin_sb")
            for r in range(NCORES):
                sl = slice(r * SS, (r + 1) * SS)
                nc.sync.dma_start(out=cos_sb[:, sl],
                                  in_=xg_out[r, DIM:DIM + 96, :])
                nc.sync.dma_start(out=sin_sb[:, sl],
                                  in_=xg_out[r, DIM + 96:DIM + 192, :])
            bq_sb = const.tile([96, 2], F32, name="bq_sb")
            nc.sync.dma_start(out=bq_sb[:], in_=bqd.ap())
            bk_sb = const.tile([96, 2], F32, name="bk_sb")
            nc.sync.dma_start(out=bk_sb[:], in_=bkd.ap())
            bv_sb = const.tile([96, 2], F32, name="bv_sb")
            nc.sync.dma_start(out=bv_sb[:], in_=bvd.ap())
            ident = const.tile([96, 96], BF16, name="ident")
            make_identity(nc, ident[:])

            # tensors that persist from projection phase into attention
            mid = tc.alloc_tile_pool(name="mid", bufs=1)
            v1 = {}
            attn = {}
            rope_out = {}
            for h in (0, 1):
                v1[h] = mid.tile([128, NKC * 128], BF16, name=f"v1_{h}",
                                 tag=f"v1_{h}")
                attn[h] = mid.tile([96, S], BF16, name=f"attn_{h}",
                                   tag=f"attn_{h}")
                for t in ("q", "k"):
                    rope_out[(t, h)] = mid.tile([96, S], BF16,
                                                name=f"r_{t}{h}",
                                                tag=f"r_{t}{h}")

            # ---- phase 1: projections + rope + v layout ----
            with tc.tile_pool(name="p3", bufs=1) as p3, \
                 tc.tile_pool(name="p3ps", bufs=8, space="PSUM") as p3ps:
                wq_sb = p3.tile([128, KC * 192], BF16, name="wq_sb")
                nc.sync.dma_start(out=wq_sb[:], in_=wqd.ap())
                wk_sb = p3.tile([128, KC * 192], BF16, name="wk_sb")
                nc.sync.dma_start(out=wk_sb[:], in_=wkd.ap())
                wv_sb = p3.tile([128, KC * 192], BF16, name="wv_sb")
                nc.sync.dma_start(out=wv_sb[:], in_=wvd.ap())
                xt = []
                for kc in range(KC):
                    t_ = p3.tile([128, S], BF16, name=f"xt{kc}", tag="xt",
                                 bufs=KC)
                    for r in range(NCORES):
                        nc.sync.dma_start(
                            out=t_[:, r * SS:(r + 1) * SS],
                            in_=xg_out[r, kc * 128:(kc + 1) * 128, :])
                    xt.append(t_)

                def project(wsb, bsb, h, dest):
                    ps = [p3ps.tile([96, 512], F32, name=f"ps{n}", tag="proj",
                                    bufs=6)
                          for n in range(NQ)]
                    for kc in range(KC):
                        lhs = wsb[:, kc * 192 + h * 96: kc * 192 + (h + 1) * 96]
                        for n in range(NQ):
                            nc.tensor.matmul(
                                ps[n][:], lhs,
                                xt[kc][:, n * 512:(n + 1) * 512],
                                start=(kc == 0), stop=(kc == KC - 1))
                    for n in range(NQ):
                        nc.vector.tensor_scalar_add(
                            dest[:, n * 512:(n + 1) * 512], ps[n][:],
                            bsb[:, h:h + 1])

                for h in (0, 1):
                    for (t, wsb, bsb) in (("q", wq_sb, bq_sb),
                                          ("k", wk_sb, bk_sb)):
                        pre = p3.tile([96, S], BF16, name=f"pre_{t}{h}",
                                      tag="pre", bufs=2)
                        project(wsb, bsb, h, pre)
                        # rope: dst = pre*cosF + swap(pre)*sinF
                        sw = p3.tile([96, S], BF16, name=f"sw_{t}{h}",
                                     tag="sw", bufs=1)
                        nc.sync.dma_start(out=sw[0:32, :], in_=pre[0:32, :])
                        nc.sync.dma_start(out=sw[32:64, :], in_=pre[64:96, :])
                        nc.sync.dma_start(out=sw[64:96, :], in_=pre[32:64, :])
                        t1 = p3.tile([96, S], BF16, name=f"t1_{t}{h}",
                                     tag="t1", bufs=1)
                        nc.vector.tensor_tensor(t1[:], pre[:], cos_sb[:],
                                                OP.mult)
                        t2 = p3.tile([96, S], BF16, name=f"t2_{t}{h}",
                                     tag="t2", bufs=1)
                        nc.vector.tensor_tensor(t2[:], sw[:], sin_sb[:],
                                                OP.mult)
                        nc.vector.tensor_tensor(rope_out[(t, h)][:], t1[:],
                                                t2[:], OP.add)
                    vt_h = p3.tile([96, S], BF16, name=f"vt{h}", tag="vt",
                                   bufs=1)
                    project(wv_sb, bv_sb, h, vt_h)
                    for kc2 in range(NKC):
                        tr = p3ps.tile([128, 96], BF16,
                                       name=f"tr{h}_{kc2}", tag="tr",
                                       bufs=2)
                        nc.tensor.transpose(
                            tr[:], vt_h[:, kc2 * 128:(kc2 + 1) * 128],
                            ident[:])
                        nc.vector.tensor_copy(
                            v1[h][:, kc2 * 128: kc2 * 128 + 96], tr[:])
                    ones_ap = v1[h].rearrange("p (k c) -> p k c", c=128)
                    nc.gpsimd.memset(ones_ap[:, :, 96:97], 1.0)

            # ---- phase 2: attention ----
            with tc.tile_pool(name="p5", bufs=1) as p5, \
                 tc.tile_pool(name="p5ps", bufs=2, space="PSUM") as ps_s, \
                 tc.tile_pool(name="pvps", bufs=2, space="PSUM") as ps_pv:
                for (h, qb) in [(h, qb) for h in (0, 1) for qb in range(NQB)]:
                    qT = rope_out[("q", h)]
                    kT = rope_out[("k", h)]
                    q0 = qb * QB
                    pv0 = ps_pv.tile([97, 512], F32, name=f"pv0_{h}{qb}",
                                     tag="pv0", bufs=2)
                    pv1 = ps_pv.tile([97, 512], F32, name=f"pv1_{h}{qb}",
                                     tag="pv1", bufs=2)
                    for kc2 in range(NKC):
                        sps = ps_s.tile([128, 1024], F32,
                                        name=f"s_{h}{qb}_{kc2}", tag="s",
                                        bufs=2)
                        lhs_k = kT[:, kc2 * 128:(kc2 + 1) * 128]
                        nc.tensor.matmul(sps[:, 0:512], lhs_k,
                                         qT[:, q0: q0 + 512])
                        nc.tensor.matmul(sps[:, 512:1024], lhs_k,
                                         qT[:, q0 + 512: q0 + 1024])
                        pt = p5.tile([128, 1024], BF16,
                                     name=f"pt_{h}{qb}_{kc2}", tag="pt",
                                     bufs=3)
                        nc.scalar.activation(pt[:], sps[:], AF.Exp,
                                             scale=SCALE)
                        lhs_v = v1[h][:, kc2 * 128: kc2 * 128 + 97]
                        nc.tensor.matmul(pv0[:], lhs_v, pt[:, 0:512],
                                         start=(kc2 == 0),
                                         stop=(kc2 == NKC - 1))
                        nc.tensor.matmul(pv1[:], lhs_v, pt[:, 512:1024],
                                         start=(kc2 == 0),
                                         stop=(kc2 == NKC - 1))
                    for j, pv in enumerate((pv0, pv1)):
                        den = p5.tile([97, 512], F32, name=f"den{h}{qb}{j}",
                                      tag="den", bufs=2)
                        nc.vector.tensor_copy(den[96:97, :], pv[96:97, :])
                        dn0 = p5.tile([1, 512], F32, name=f"dn0{h}{qb}{j}",
                                      tag="dn0", bufs=2)
                        nc.sync.dma_start(out=dn0[:], in_=den[96:97, :])
                        rec = p5.tile([1, 512], F32, name=f"rec{h}{qb}{j}",
                                      tag="rec", bufs=2)
                        nc.vector.reciprocal(rec[:], dn0[:])
                        rb = p5.tile([96, 512], F32, name=f"rb{h}{qb}{j}",
                                     tag="rb", bufs=2)
                        nc.gpsimd.partition_broadcast(rb[:], rec[:])
                        nc.vector.tensor_tensor(
                            attn[h][:, q0 + j * 512: q0 + (j + 1) * 512],
                            pv[0:96, :], rb[:], OP.mult)

            # ---- collective 2: gather per-head attention outputs ----
            for h in (0, 1):
                nc.sync.dma_start(out=ag_in[h * 96:(h + 1) * 96, :],
                                  in_=attn[h][:])
            nc.gpsimd.collective_compute(
                "AllGather", mybir.AluOpType.bypass, replica_groups=groups,
                ins=[ag_in[:]], outs=[ag_out[:]])

            # ---- phase 3: output projection (my 192 columns of o) ----
            with tc.tile_pool(name="p6", bufs=1) as p6, \
                 tc.tile_pool(name="p6ps", bufs=4, space="PSUM") as p6ps:
                wo_sb = p6.tile([128, KC * 192], BF16, name="wo_sb")
                nc.sync.dma_start(out=wo_sb[:], in_=wod.ap())
                at = []
                for kc in range(KC):
                    t_ = p6.tile([128, S], BF16, name=f"at{kc}", tag="at",
                                 bufs=KC)
                    g0, g1 = 128 * kc, 128 * (kc + 1)
                    g = g0
                    while g < g1:
                        r = g // 192
                        take = min(g1, 192 * (r + 1)) - g
                        nc.sync.dma_start(
                            out=t_[g - g0: g - g0 + take, :],
                            in_=ag_out[r, g - 192 * r: g - 192 * r + take, :])
                        g += take
                    at.append(t_)
                for n in range(NQ):
                    for g in (0, 1):
                        po = p6ps.tile([96, 512], F32, name=f"po{n}_{g}",
                                       tag="po")
                        for kc in range(KC):
                            nc.tensor.matmul(
                                po[:],
                                wo_sb[:, kc * 192 + g * 96:
                                      kc * 192 + (g + 1) * 96],
                                at[kc][:, n * 512:(n + 1) * 512],
                                start=(kc == 0), stop=(kc == KC - 1))
                        ot = p6.tile([96, 512], F16, name=f"ot{n}_{g}",
                                     tag="ot", bufs=4)
                        if (2 * n + g) % 2:
                            nc.scalar.activation(ot[:], po[:], AF.Copy)
                        else:
                            nc.vector.tensor_copy(ot[:], po[:])
                        nc.sync.dma_start(
                            out=outd.ap()[g * 96:(g + 1) * 96,
                                          n * 512:(n + 1) * 512],
                            in_=ot[:])
            mid.release()
            const.release()

    nc.compile()
    return nc


def _get_nc():
    if "nc" not in _STATE:
        _STATE["nc"] = _build_nc()
    return _STATE["nc"]


def make_in_maps(x, wq, bq, wk, bk, wv, bv, wo, bo, freqs_cos, freqs_sin,
                 h, w):
    """Host-side shard prep: pure indexing/casting, returns per-core in_maps."""
    assert int(h) == GH and int(w) == GW
    x = np.asarray(x, np.float32)
    wq = np.asarray(wq, np.float32)
    wk = np.asarray(wk, np.float32)
    wv = np.asarray(wv, np.float32)
    wo = np.asarray(wo, np.float32)
    bq = np.asarray(bq, np.float32)
    bk = np.asarray(bk, np.float32)
    bv = np.asarray(bv, np.float32)
    fc = np.asarray(freqs_cos, np.float32)
    fs = np.asarray(freqs_sin, np.float32)

    perm = np.asarray(_PERM)

    # rope tables in the permuted row basis
    tpos = np.arange(S)
    gh = tpos // GW
    gw = tpos % GW
    c32 = np.empty((32, S), np.float32)
    s32 = np.empty((32, S), np.float32)
    c32[0:16] = fc[gh, 16:32].T
    c32[16:32] = fc[gw, 32:48].T
    s32[0:16] = fs[gh, 16:32].T
    s32[16:32] = fs[gw, 32:48].T
    cosF = np.ones((96, S), np.float32)
    cosF[32:64] = c32
    cosF[64:96] = c32
    sinF = np.zeros((96, S), np.float32)
    sinF[32:64] = -s32
    sinF[64:96] = s32
    cosF = cosF.astype(BF16N)
    sinF = sinF.astype(BF16N)

    xT = np.ascontiguousarray(x[0].astype(BF16N).T)   # [DIM, S]

    def tile_w(wc):
        # [1536, 192] -> [128, KC*192] with col block kc = rows kc*128..+128
        return np.ascontiguousarray(
            wc.reshape(KC, 128, 192).transpose(1, 0, 2).reshape(128, KC * 192)
        ).astype(BF16N)

    in_maps = []
    for c in range(NCORES):
        h0, h1 = 2 * c, 2 * c + 1
        sl = slice(c * SS, (c + 1) * SS)
        xcs = np.empty((XR, SS), BF16N)
        xcs[0:DIM] = xT[:, sl]
        xcs[DIM:DIM + 96] = cosF[:, sl]
        xcs[DIM + 96:DIM + 192] = sinF[:, sl]
        qk_cols = np.concatenate([h0 * HD + perm, h1 * HD + perm])
        v_cols = np.arange(h0 * HD, (h1 + 1) * HD)
        bq_c = np.stack([bq[h0 * HD + perm], bq[h1 * HD + perm]], axis=1)
        bk_c = np.stack([bk[h0 * HD + perm], bk[h1 * HD + perm]], axis=1)
        bv_c = np.stack([bv[v_cols[:HD]], bv[v_cols[HD:]]], axis=1)
        in_maps.append({
            "xcs_s": xcs,
            "wq_t": tile_w(wq[:, qk_cols]),
            "wk_t": tile_w(wk[:, qk_cols]),
            "wv_t": tile_w(wv[:, v_cols]),
            "wo_t": tile_w(wo[:, 2 * HD * c: 2 * HD * (c + 1)]),
            "bq_t": np.ascontiguousarray(bq_c, dtype=np.float32),
            "bk_t": np.ascontiguousarray(bk_c, dtype=np.float32),
            "bv_t": np.ascontiguousarray(bv_c, dtype=np.float32),
        })
    return in_maps


def assemble_output(parts, bo):
    """parts: list of 8 [192, S] fp16 arrays (row-slices of o^T)."""
    oT = np.concatenate([np.asarray(p) for p in parts], axis=0)  # [DIM, S]
    out = oT.astype(np.float32).T + np.asarray(bo, np.float32)[None, :]
    return out[None]


def kernel(x, wq, bq, wk, bk, wv, bv, wo, bo, freqs_cos, freqs_sin, h, w):
    from concourse.bass_utils import run_bass_kernel_spmd

    nc = _get_nc()
    in_maps = make_in_maps(x, wq, bq, wk, bk, wv, bv, wo, bo,
                           freqs_cos, freqs_sin, h, w)
    res = run_bass_kernel_spmd(nc, in_maps, core_ids=list(range(NCORES)))
    parts = [res.results[c]["o_s"] for c in range(NCORES)]
    return assemble_output(parts, bo)


# revision 3
# speedup vs baseline: 1.0243x; 1.0243x over previous
"""DeRA attention (2D-rope attention) Trainium2 kernel, 8-core head-parallel
with on-device collectives to minimize host<->device traffic.

Host I/O per core (the dominant cost under this harness):
 - up:   xcs_s [1728, 384] bf16  = this core's 1/8 sequence-slice of x^T
         stacked with its slice of the rope cos/sin tables (1.33 MB)
         wq/wk/wv head-shard [128, 12*192] bf16 (0.59 MB each)
         wo column-shard    [128, 12*192] bf16 (0.59 MB)
         biases [96, 2] f32 (tiny)
 - down: o8 [192, 3072] int8 + osc [192, 1] f32 = this core's 192-row
         slice of o^T, int8-quantized with one fp32 scale per row
         (0.59 MB; max quantization error absmax/254 per row).

Device dataflow:
 1. AllGather the x^T/table slices (5 chunks: tables first, then three
    128-row groups at a time) so projection matmuls on early chunks
    overlap the remaining gathers.
 2. Per-core head-parallel compute (2 of 16 heads): q/k/v projections,
    2D-rope (baked cos/sin mul + partition-block swap), dense attention
    with exp-sum-trick softmax (ones row appended to V gives the
    denominator inside the PV accumulation).
 3. AllGather each finished head's attention output (bf16) as soon as
    its rounds complete (head-0 gather overlaps head-1 compute); each
    core then computes its own 192 columns of the output projection in
    fp32 PSUM, row-quantizes to int8, and downloads.
Host sums nothing: it concatenates+dequantizes the 8 row-slices and
adds bo.
"""

import sys

if "/opt/trn_rl_repo" not in sys.path:
    sys.path.insert(0, "/opt/trn_rl_repo")

import numpy as np
import ml_dtypes

BF16N = ml_dtypes.bfloat16

# Problem config (hardcoded per spec)
S = 3072
DIM = 1536
NH = 16
HD = 96
NCORES = 8
GH, GW = 48, 64
SS = S // NCORES         # 384 sequence positions per core
KC = DIM // 128          # 12 contraction chunks of 128
NQ = S // 512            # 6
NKC = S // 128           # 24 key chunks
QB = 1024                # q block (columns per attention round)
NQB = S // QB            # 3
XR = DIM + 2 * HD        # 1728 rows in the fused x+tables upload
SCALE = 1.0 / float(np.sqrt(HD))

# per-head column permutation: [id evens, id odds, rot evens, rot odds]
_PERM = (
    [2 * j for j in range(16)]
    + [2 * j + 1 for j in range(16)]
    + [2 * j for j in range(16, 48)]
    + [2 * j + 1 for j in range(16, 48)]
)

# AllGather-1 chunks: (row_start, row_end) of the [XR, SS] upload.
# Tables first (rope needs them early), then 3 x-row-chunks per gather.
_AG1_CHUNKS = [(DIM, XR)] + [(kc0 * 128, (kc0 + 3) * 128)
                             for kc0 in range(0, KC, 3)]

_STATE = {}


def _build_nc():
    import concourse.bass as bass  # noqa: F401
    import concourse.tile as tile
    from concourse import bacc, mybir
    from concourse.masks import make_identity

    BF16 = mybir.dt.bfloat16
    F32 = mybir.dt.float32
    I8 = mybir.dt.int8
    AF = mybir.ActivationFunctionType
    OP = mybir.AluOpType

    nc = bacc.Bacc("TRN2", target_bir_lowering=False, debug=False,
                   num_devices=NCORES)

    xcsd = nc.dram_tensor("xcs_s", [XR, SS], BF16, kind="ExternalInput")
    wqd = nc.dram_tensor("wq_t", [128, KC * 192], BF16, kind="ExternalInput")
    wkd = nc.dram_tensor("wk_t", [128, KC * 192], BF16, kind="ExternalInput")
    wvd = nc.dram_tensor("wv_t", [128, KC * 192], BF16, kind="ExternalInput")
    wod = nc.dram_tensor("wo_t", [128, KC * 192], BF16, kind="ExternalInput")
    bqd = nc.dram_tensor("bq_t", [96, 2], F32, kind="ExternalInput")
    bkd = nc.dram_tensor("bk_t", [96, 2], F32, kind="ExternalInput")
    bvd = nc.dram_tensor("bv_t", [96, 2], F32, kind="ExternalInput")
    outd = nc.dram_tensor("o8", [2 * HD, S], I8, kind="ExternalOutput")
    oscd = nc.dram_tensor("osc", [2 * HD, 1], F32, kind="ExternalOutput")

    groups = [list(range(NCORES))]

    with tile.TileContext(nc) as tc:
        with tc.tile_pool(name="dram", bufs=1, space="DRAM") as dram:
            # ---- collective 1 (chunked): gather x^T + rope tables ----
            xg_out = {}
            for (r0, r1) in _AG1_CHUNKS:
                gi = dram.tile([r1 - r0, SS], BF16, name=f"xgi_{r0}",
                               tag=f"xgi_{r0}")
                nc.sync.dma_start(out=gi[:], in_=xcsd.ap()[r0:r1, :])
                go = dram.tile([NCORES, r1 - r0, SS], BF16, name=f"xgo_{r0}",
                               tag=f"xgo_{r0}", addr_space="Shared")
                nc.gpsimd.collective_compute(
                    "AllGather", mybir.AluOpType.bypass, replica_groups=groups,
                    ins=[gi[:]], outs=[go[:]])
                xg_out[r0] = go

            def xg_rows(row0, nrows, dst):
                """DMA [nrows, S] of the gathered x^T/table matrix into dst,
                reassembling the 8 rank-slices of the right chunk."""
                for (r0, r1) in _AG1_CHUNKS:
                    if r0 <= row0 < r1:
                        assert row0 + nrows <= r1
                        go = xg_out[r0]
                        for r in range(NCORES):
                            nc.sync.dma_start(
                                out=dst[:, r * SS:(r + 1) * SS],
                                in_=go[r, row0 - r0:row0 - r0 + nrows, :])
                        return
                raise AssertionError(row0)

            ag_h = {}
            for h in (0, 1):
                ag_h[h] = (dram.tile([96, S], BF16, name=f"agi{h}"),
                           dram.tile([NCORES, 96, S], BF16, name=f"ago{h}",
                                     addr_space="Shared"))

            const = tc.alloc_tile_pool(name="const", bufs=1)
            cos_sb = const.tile([96, S], BF16, name="cos_sb")
            xg_rows(DIM, 96, cos_sb)
            sin_sb = const.tile([96, S], BF16, name="sin_sb")
            xg_rows(DIM + 96, 96, sin_sb)
            bq_sb = const.tile([96, 2], F32, name="bq_sb")
            nc.sync.dma_start(out=bq_sb[:], in_=bqd.ap())
            bk_sb = const.tile([96, 2], F32, name="bk_sb")
            nc.sync.dma_start(out=bk_sb[:], in_=bkd.ap())
            bv_sb = const.tile([96, 2], F32, name="bv_sb")
            nc.sync.dma_start(out=bv_sb[:], in_=bvd.ap())
            ident = const.tile([96, 96], BF16, name="ident")
            make_identity(nc, ident[:])

            # tensors that persist from projection phase into attention
            mid = tc.alloc_tile_pool(name="mid", bufs=1)
            v1 = {}
            attn = {}
            rope_out = {}
            for h in (0, 1):
                v1[h] = mid.tile([128, NKC * 128], BF16, name=f"v1_{h}",
                                 tag=f"v1_{h}")
                attn[h] = mid.tile([96, S], BF16, name=f"attn_{h}",
                                   tag=f"attn_{h}")
                for t in ("q", "k"):
                    rope_out[(t, h)] = mid.tile([96, S], BF16,
                                                name=f"r_{t}{h}",
                                                tag=f"r_{t}{h}")

            # ---- phase 1: projections + rope + v layout ----
            with tc.tile_pool(name="p3", bufs=1) as p3, \
                 tc.tile_pool(name="p3ps", bufs=8, space="PSUM") as p3ps:
                wq_sb = p3.tile([128, KC * 192], BF16, name="wq_sb")
                nc.sync.dma_start(out=wq_sb[:], in_=wqd.ap())
                wk_sb = p3.tile([128, KC * 192], BF16, name="wk_sb")
                nc.sync.dma_start(out=wk_sb[:], in_=wkd.ap())
                wv_sb = p3.tile([128, KC * 192], BF16, name="wv_sb")
                nc.sync.dma_start(out=wv_sb[:], in_=wvd.ap())
                xt = []
                for kc in range(KC):
                    t_ = p3.tile([128, S], BF16, name=f"xt{kc}", tag="xt",
                                 bufs=KC)
                    xg_rows(kc * 128, 128, t_)
                    xt.append(t_)

                def project(wsb, bsb, h, dest):
                    ps = [p3ps.tile([96, 512], F32, name=f"ps{n}", tag="proj",
                                    bufs=6)
                          for n in range(NQ)]
                    for kc in range(KC):
                        lhs = wsb[:, kc * 192 + h * 96: kc * 192 + (h + 1) * 96]
                        for n in range(NQ):
                            nc.tensor.matmul(
                                ps[n][:], lhs,
                                xt[kc][:, n * 512:(n + 1) * 512],
                                start=(kc == 0), stop=(kc == KC - 1))
                    for n in range(NQ):
                        nc.vector.tensor_scalar_add(
                            dest[:, n * 512:(n + 1) * 512], ps[n][:],
                            bsb[:, h:h + 1])

                for h in (0, 1):
                    for (t, wsb, bsb) in (("q", wq_sb, bq_sb),
                                          ("k", wk_sb, bk_sb)):
                        pre = p3.tile([96, S], BF16, name=f"pre_{t}{h}",
                                      tag="pre", bufs=2)
                        project(wsb, bsb, h, pre)
                        # rope: dst = pre*cosF + swap(pre)*sinF
                        sw = p3.tile([96, S], BF16, name=f"sw_{t}{h}",
                                     tag="sw", bufs=1)
                        nc.sync.dma_start(out=sw[0:32, :], in_=pre[0:32, :])
                        nc.sync.dma_start(out=sw[32:64, :], in_=pre[64:96, :])
                        nc.sync.dma_start(out=sw[64:96, :], in_=pre[32:64, :])
                        t1 = p3.tile([96, S], BF16, name=f"t1_{t}{h}",
                                     tag="t1", bufs=1)
                        nc.vector.tensor_tensor(t1[:], pre[:], cos_sb[:],
                                                OP.mult)
                        t2 = p3.tile([96, S], BF16, name=f"t2_{t}{h}",
                                     tag="t2", bufs=1)
                        nc.vector.tensor_tensor(t2[:], sw[:], sin_sb[:],
                                                OP.mult)
                        nc.vector.tensor_tensor(rope_out[(t, h)][:], t1[:],
                                                t2[:], OP.add)
                    vt_h = p3.tile([96, S], BF16, name=f"vt{h}", tag="vt",
                                   bufs=1)
                    project(wv_sb, bv_sb, h, vt_h)
                    for kc2 in range(NKC):
                        tr = p3ps.tile([128, 96], BF16,
                                       name=f"tr{h}_{kc2}", tag="tr",
                                       bufs=2)
                        nc.tensor.transpose(
                            tr[:], vt_h[:, kc2 * 128:(kc2 + 1) * 128],
                            ident[:])
                        nc.vector.tensor_copy(
                            v1[h][:, kc2 * 128: kc2 * 128 + 96], tr[:])
                    ones_ap = v1[h].rearrange("p (k c) -> p k c", c=128)
                    nc.gpsimd.memset(ones_ap[:, :, 96:97], 1.0)

            # ---- phase 2: attention (+ per-head gather as it finishes) ----
            with tc.tile_pool(name="p5", bufs=1) as p5, \
                 tc.tile_pool(name="p5ps", bufs=2, space="PSUM") as ps_s, \
                 tc.tile_pool(name="pvps", bufs=2, space="PSUM") as ps_pv:
                for h in (0, 1):
                    for qb in range(NQB):
                        qT = rope_out[("q", h)]
                        kT = rope_out[("k", h)]
                        q0 = qb * QB
                        pv0 = ps_pv.tile([97, 512], F32, name=f"pv0_{h}{qb}",
                                         tag="pv0", bufs=2)
                        pv1 = ps_pv.tile([97, 512], F32, name=f"pv1_{h}{qb}",
                                         tag="pv1", bufs=2)
                        for kc2 in range(NKC):
                            sps = ps_s.tile([128, 1024], F32,
                                            name=f"s_{h}{qb}_{kc2}", tag="s",
                                            bufs=2)
                            lhs_k = kT[:, kc2 * 128:(kc2 + 1) * 128]
                            nc.tensor.matmul(sps[:, 0:512], lhs_k,
                                             qT[:, q0: q0 + 512])
                            nc.tensor.matmul(sps[:, 512:1024], lhs_k,
                                             qT[:, q0 + 512: q0 + 1024])
                            pt = p5.tile([128, 1024], BF16,
                                         name=f"pt_{h}{qb}_{kc2}", tag="pt",
                                         bufs=3)
                            nc.scalar.activation(pt[:], sps[:], AF.Exp,
                                                 scale=SCALE)
                            lhs_v = v1[h][:, kc2 * 128: kc2 * 128 + 97]
                            nc.tensor.matmul(pv0[:], lhs_v, pt[:, 0:512],
                                             start=(kc2 == 0),
                                             stop=(kc2 == NKC - 1))
                            nc.tensor.matmul(pv1[:], lhs_v, pt[:, 512:1024],
                                             start=(kc2 == 0),
                                             stop=(kc2 == NKC - 1))
                        for j, pv in enumerate((pv0, pv1)):
                            den = p5.tile([97, 512], F32,
                                          name=f"den{h}{qb}{j}",
                                          tag="den", bufs=2)
                            nc.vector.tensor_copy(den[96:97, :], pv[96:97, :])
                            dn0 = p5.tile([1, 512], F32, name=f"dn0{h}{qb}{j}",
                                          tag="dn0", bufs=2)
                            nc.sync.dma_start(out=dn0[:], in_=den[96:97, :])
                            rec = p5.tile([1, 512], F32, name=f"rec{h}{qb}{j}",
                                          tag="rec", bufs=2)
                            nc.vector.reciprocal(rec[:], dn0[:])
                            rb = p5.tile([96, 512], F32, name=f"rb{h}{qb}{j}",
                                         tag="rb", bufs=2)
                            nc.gpsimd.partition_broadcast(rb[:], rec[:])
                            nc.vector.tensor_tensor(
                                attn[h][:, q0 + j * 512: q0 + (j + 1) * 512],
                                pv[0:96, :], rb[:], OP.mult)
                    # gather this head's output while the next head computes
                    agi, _ago = ag_h[h]
                    nc.sync.dma_start(out=agi[:], in_=attn[h][:])
                    nc.gpsimd.collective_compute(
                        "AllGather", mybir.AluOpType.bypass,
                        replica_groups=groups,
                        ins=[agi[:]], outs=[_ago[:]])

            # ---- phase 3: output projection (my 192 columns of o),
            #      int8 row-quantized download ----
            with tc.tile_pool(name="p6", bufs=1) as p6, \
                 tc.tile_pool(name="p6ps", bufs=4, space="PSUM") as p6ps:
                wo_sb = p6.tile([128, KC * 192], BF16, name="wo_sb")
                nc.sync.dma_start(out=wo_sb[:], in_=wod.ap())
                at = []
                for kc in range(KC):
                    t_ = p6.tile([128, S], BF16, name=f"at{kc}", tag="at",
                                 bufs=KC)
                    R, R1 = 128 * kc, 128 * (kc + 1)
                    while R < R1:
                        head = R // 96
                        take = min(R1 - R, 96 - R % 96)
                        nc.sync.dma_start(
                            out=t_[R - 128 * kc: R - 128 * kc + take, :],
                            in_=ag_h[head % 2][1][head // 2,
                                                  R % 96: R % 96 + take, :])
                        R += take
                    at.append(t_)
                orow = {}
                for g in (0, 1):
                    orow[g] = p6.tile([96, S], F32, name=f"orow{g}",
                                      tag=f"orow{g}")
                for n in range(NQ):
                    for g in (0, 1):
                        po = p6ps.tile([96, 512], F32, name=f"po{n}_{g}",
                                       tag="po")
                        for kc in range(KC):
                            nc.tensor.matmul(
                                po[:],
                                wo_sb[:, kc * 192 + g * 96:
                                      kc * 192 + (g + 1) * 96],
                                at[kc][:, n * 512:(n + 1) * 512],
                                start=(kc == 0), stop=(kc == KC - 1))
                        if (2 * n + g) % 2:
                            nc.scalar.activation(
                                orow[g][:, n * 512:(n + 1) * 512], po[:],
                                AF.Copy)
                        else:
                            nc.vector.tensor_copy(
                                orow[g][:, n * 512:(n + 1) * 512], po[:])
                for g in (0, 1):
                    rowmax = p6.tile([96, 1], F32, name=f"rmax{g}",
                                     tag=f"rmax{g}")
                    nc.vector.tensor_reduce(
                        rowmax[:], orow[g][:], axis=mybir.AxisListType.X,
                        op=OP.max, apply_absolute_value=True)
                    rinv = p6.tile([96, 1], F32, name=f"rinv{g}",
                                   tag=f"rinv{g}")
                    nc.vector.reciprocal(rinv[:], rowmax[:])
                    qinv = p6.tile([96, 1], F32, name=f"qinv{g}",
                                   tag=f"qinv{g}")
                    nc.scalar.activation(qinv[:], rinv[:], AF.Copy,
                                         scale=127.0)
                    sc = p6.tile([96, 1], F32, name=f"sc{g}", tag=f"sc{g}")
                    nc.scalar.activation(sc[:], rowmax[:], AF.Copy,
                                         scale=1.0 / 127.0)
                    nc.sync.dma_start(out=oscd.ap()[g * 96:(g + 1) * 96, :],
                                      in_=sc[:])
                    q8 = p6.tile([96, S], I8, name=f"q8{g}", tag=f"q8{g}")
                    nc.vector.tensor_scalar_mul(q8[:], orow[g][:],
                                                qinv[:, 0:1])
                    nc.sync.dma_start(out=outd.ap()[g * 96:(g + 1) * 96, :],
                                      in_=q8[:])
            mid.release()
            const.release()

    nc.compile()
    return nc


def _get_nc():
    if "nc" not in _STATE:
        _STATE["nc"] = _build_nc()
    return _STATE["nc"]


def make_in_maps(x, wq, bq, wk, bk, wv, bv, wo, bo, freqs_cos, freqs_sin,
                 h, w):
    """Host-side shard prep: pure indexing/casting, returns per-core in_maps."""
    assert int(h) == GH and int(w) == GW
    x = np.asarray(x, np.float32)
    wq = np.asarray(wq, np.float32)
    wk = np.asarray(wk, np.float32)
    wv = np.asarray(wv, np.float32)
    wo = np.asarray(wo, np.float32)
    bq = np.asarray(bq, np.float32)
    bk = np.asarray(bk, np.float32)
    bv = np.asarray(bv, np.float32)
    fc = np.asarray(freqs_cos, np.float32)
    fs = np.asarray(freqs_sin, np.float32)

    perm = np.asarray(_PERM)

    # rope tables in the permuted row basis
    tpos = np.arange(S)
    gh = tpos // GW
    gw = tpos % GW
    c32 = np.empty((32, S), np.float32)
    s32 = np.empty((32, S), np.float32)
    c32[0:16] = fc[gh, 16:32].T
    c32[16:32] = fc[gw, 32:48].T
    s32[0:16] = fs[gh, 16:32].T
    s32[16:32] = fs[gw, 32:48].T
    cosF = np.ones((96, S), np.float32)
    cosF[32:64] = c32
    cosF[64:96] = c32
    sinF = np.zeros((96, S), np.float32)
    sinF[32:64] = -s32
    sinF[64:96] = s32
    cosF = cosF.astype(BF16N)
    sinF = sinF.astype(BF16N)

    xT = np.ascontiguousarray(x[0].astype(BF16N).T)   # [DIM, S]

    def tile_w(wc):
        # [1536, 192] -> [128, KC*192] with col block kc = rows kc*128..+128
        return np.ascontiguousarray(
            wc.reshape(KC, 128, 192).transpose(1, 0, 2).reshape(128, KC * 192)
        ).astype(BF16N)

    in_maps = []
    for c in range(NCORES):
        h0, h1 = 2 * c, 2 * c + 1
        sl = slice(c * SS, (c + 1) * SS)
        xcs = np.empty((XR, SS), BF16N)
        xcs[0:DIM] = xT[:, sl]
        xcs[DIM:DIM + 96] = cosF[:, sl]
        xcs[DIM + 96:DIM + 192] = sinF[:, sl]
        qk_cols = np.concatenate([h0 * HD + perm, h1 * HD + perm])
        v_cols = np.arange(h0 * HD, (h1 + 1) * HD)
        bq_c = np.stack([bq[h0 * HD + perm], bq[h1 * HD + perm]], axis=1)
        bk_c = np.stack([bk[h0 * HD + perm], bk[h1 * HD + perm]], axis=1)
        bv_c = np.stack([bv[v_cols[:HD]], bv[v_cols[HD:]]], axis=1)
        in_maps.append({
            "xcs_s": xcs,
            "wq_t": tile_w(wq[:, qk_cols]),
            "wk_t": tile_w(wk[:, qk_cols]),
            "wv_t": tile_w(wv[:, v_cols]),
            "wo_t": tile_w(wo[:, 2 * HD * c: 2 * HD * (c + 1)]),
            "bq_t": np.ascontiguousarray(bq_c, dtype=np.float32),
            "bk_t": np.ascontiguousarray(bk_c, dtype=np.float32),
            "bv_t": np.ascontiguousarray(bv_c, dtype=np.float32),
        })
    return in_maps


def assemble_output(parts, bo):
    """parts: list of 8 (o8 [192, S] int8, osc [192, 1] f32) tuples."""
    o8 = np.concatenate([np.asarray(p[0]) for p in parts], axis=0)
    osc = np.concatenate([np.asarray(p[1]) for p in parts], axis=0)
    oT = o8.astype(np.float32) * osc                     # [DIM, S]
    out = oT.T + np.asarray(bo, np.float32)[None, :]
    return out[None]


def kernel(x, wq, bq, wk, bk, wv, bv, wo, bo, freqs_cos, freqs_sin, h, w):
    from concourse.bass_utils import run_bass_kernel_spmd

    nc = _get_nc()
    in_maps = make_in_maps(x, wq, bq, wk, bk, wv, bv, wo, bo,
                           freqs_cos, freqs_sin, h, w)
    res = run_bass_kernel_spmd(nc, in_maps, core_ids=list(range(NCORES)))
    parts = [(res.results[c]["o8"], res.results[c]["osc"])
             for c in range(NCORES)]
    return assemble_output(parts, bo)


# revision 12
# speedup vs baseline: 1.1294x; 1.1025x over previous
"""DeRA attention (2D-rope attention) Trainium2 kernel, 8-core head-parallel
with on-device collectives to minimize host<->device traffic.

Host I/O per core (the dominant cost under this harness):
 - up:   xcs_s [1728, 384] bf16  = this core's 1/8 sequence-slice of x^T
         stacked with its slice of the rope cos/sin tables (1.33 MB)
         wq/wk/wv head-shard [128, 12*192] bf16 (0.59 MB each)
         wo column-shard    [128, 12*192] bf16 (0.59 MB)
         biases [96, 2] f32 (tiny)
 - down: o8 [192, 3072] int8 + osc [192, 1] f32 = this core's 192-row
         slice of o^T, int8-quantized with one fp32 scale per row
         (0.59 MB; max quantization error absmax/254 per row).

Device dataflow:
 1. AllGather the x^T/table slices (5 chunks: tables first, then three
    128-row groups at a time) so projection matmuls on early chunks
    overlap the remaining gathers.
 2. Per-core head-parallel compute (2 of 16 heads): q/k/v projections,
    2D-rope (baked cos/sin mul + partition-block swap), dense attention
    with exp-sum-trick softmax (ones row appended to V gives the
    denominator inside the PV accumulation).
 3. AllGather each finished head's attention output (bf16) as soon as
    its rounds complete (head-0 gather overlaps head-1 compute); each
    core then computes its own 192 columns of the output projection in
    fp32 PSUM, row-quantizes to int8, and downloads.
Host sums nothing: it concatenates+dequantizes the 8 row-slices and
adds bo.
"""

import sys

if "/opt/trn_rl_repo" not in sys.path:
    sys.path.insert(0, "/opt/trn_rl_repo")

import numpy as np
import ml_dtypes

BF16N = ml_dtypes.bfloat16

# Problem config (hardcoded per spec)
S = 3072
DIM = 1536
NH = 16
HD = 96
NCORES = 8
GH, GW = 48, 64
SS = S // NCORES         # 384 sequence positions per core
KC = DIM // 128          # 12 contraction chunks of 128
NQ = S // 512            # 6
NKC = S // 128           # 24 key chunks
QB = 1024                # q block (columns per attention round)
NQB = S // QB            # 3
XR = DIM + 2 * HD        # 1728 rows in the fused x+tables upload
SCALE = 1.0 / float(np.sqrt(HD))

# per-head column permutation: [id evens, id odds, rot evens, rot odds]
_PERM = (
    [2 * j for j in range(16)]
    + [2 * j + 1 for j in range(16)]
    + [2 * j for j in range(16, 48)]
    + [2 * j + 1 for j in range(16, 48)]
)

# AllGather-1 chunks: lists of (row_start, row_end) ranges of the [XR, SS]
# upload, packed into one gather each. Tables ride in the first chunk (rope
# needs them early); 3 chunks total amortize the per-collective floor while
# letting projection matmuls on early chunks overlap the remaining gathers.
_AG1_CHUNKS = [
    [(DIM, XR), (0, 256)],      # tables + kc 0-1
    [(256, 896)],               # kc 2-6
    [(896, DIM)],               # kc 7-11
]

_STATE = {}


def _build_nc():
    import concourse.bass as bass  # noqa: F401
    import concourse.tile as tile
    from concourse import bacc, mybir
    from concourse.masks import make_identity

    BF16 = mybir.dt.bfloat16
    F32 = mybir.dt.float32
    I8 = mybir.dt.int8
    AF = mybir.ActivationFunctionType
    OP = mybir.AluOpType

    nc = bacc.Bacc("TRN2", target_bir_lowering=False, debug=False,
                   num_devices=NCORES)

    xcsd = nc.dram_tensor("xcs_s", [XR, SS], BF16, kind="ExternalInput")
    wqd = nc.dram_tensor("wq_t", [128, KC * 192], BF16, kind="ExternalInput")
    wkd = nc.dram_tensor("wk_t", [128, KC * 192], BF16, kind="ExternalInput")
    wvd = nc.dram_tensor("wv_t", [128, KC * 192], BF16, kind="ExternalInput")
    wod = nc.dram_tensor("wo_t", [128, KC * 192], BF16, kind="ExternalInput")
    bqd = nc.dram_tensor("bq_t", [96, 2], F32, kind="ExternalInput")
    bkd = nc.dram_tensor("bk_t", [96, 2], F32, kind="ExternalInput")
    bvd = nc.dram_tensor("bv_t", [96, 2], F32, kind="ExternalInput")
    outd = nc.dram_tensor("o8", [2 * HD, S], I8, kind="ExternalOutput")
    oscd = nc.dram_tensor("osc", [2 * HD, 1], F32, kind="ExternalOutput")

    groups = [list(range(NCORES))]

    with tile.TileContext(nc) as tc:
        with tc.tile_pool(name="dram", bufs=1, space="DRAM") as dram:
            # ---- collective 1 (chunked): gather x^T + rope tables ----
            xg_out = []   # (go_tile, [(row0, row1, off_in_chunk), ...])
            for ci, ranges in enumerate(_AG1_CHUNKS):
                rows = sum(r1 - r0 for (r0, r1) in ranges)
                gi = dram.tile([rows, SS], BF16, name=f"xgi_{ci}",
                               tag=f"xgi_{ci}")
                off = 0
                offmap = []
                for (r0, r1) in ranges:
                    nc.sync.dma_start(out=gi[off:off + r1 - r0, :],
                                      in_=xcsd.ap()[r0:r1, :])
                    offmap.append((r0, r1, off))
                    off += r1 - r0
                go = dram.tile([NCORES, rows, SS], BF16, name=f"xgo_{ci}",
                               tag=f"xgo_{ci}", addr_space="Shared")
                nc.gpsimd.collective_compute(
                    "AllGather", mybir.AluOpType.bypass, replica_groups=groups,
                    ins=[gi[:]], outs=[go[:]])
                xg_out.append((go, offmap))

            def xg_rows(row0, nrows, dst):
                """DMA [nrows, S] of the gathered x^T/table matrix into dst,
                reassembling the 8 rank-slices of the right chunk."""
                for (go, offmap) in xg_out:
                    for (r0, r1, off) in offmap:
                        if r0 <= row0 < r1:
                            assert row0 + nrows <= r1
                            o = off + row0 - r0
                            for r in range(NCORES):
                                nc.sync.dma_start(
                                    out=dst[:, r * SS:(r + 1) * SS],
                                    in_=go[r, o:o + nrows, :])
                            return
                raise AssertionError(row0)

            # attention-output gathers: one per 1024-column block, covering
            # BOTH local heads, issued as soon as both heads finish the block
            # (head rounds are interleaved) so only the last gather's wire
            # time is exposed after the attention compute
            ag_c = {}
            for qb in range(NQB):
                ag_c[qb] = (
                    dram.tile([2 * 96, QB], BF16, name=f"agi{qb}"),
                    dram.tile([NCORES, 2 * 96, QB], BF16,
                              name=f"ago{qb}", addr_space="Shared"))

            const = tc.alloc_tile_pool(name="const", bufs=1)
            cos_sb = const.tile([96, S], BF16, name="cos_sb")
            xg_rows(DIM, 96, cos_sb)
            sin_sb = const.tile([96, S], BF16, name="sin_sb")
            xg_rows(DIM + 96, 96, sin_sb)
            bq_sb = const.tile([96, 2], F32, name="bq_sb")
            nc.sync.dma_start(out=bq_sb[:], in_=bqd.ap())
            bk_sb = const.tile([96, 2], F32, name="bk_sb")
            nc.sync.dma_start(out=bk_sb[:], in_=bkd.ap())
            bv_sb = const.tile([96, 2], F32, name="bv_sb")
            nc.sync.dma_start(out=bv_sb[:], in_=bvd.ap())
            ident = const.tile([96, 96], BF16, name="ident")
            make_identity(nc, ident[:])
            # wo is only needed in phase 3, but load it up front so the DMA
            # hides under phase 1 instead of gating the output projection
            wo_sb = const.tile([128, KC * 192], BF16, name="wo_sb")
            nc.sync.dma_start(out=wo_sb[:], in_=wod.ap())

            # tensors that persist from projection phase into attention
            mid = tc.alloc_tile_pool(name="mid", bufs=1)
            v1 = {}
            attn = {}
            rope_out = {}
            for h in (0, 1):
                v1[h] = mid.tile([128, NKC * 128], BF16, name=f"v1_{h}",
                                 tag=f"v1_{h}")
                attn[h] = mid.tile([96, S], BF16, name=f"attn_{h}",
                                   tag=f"attn_{h}")
                for t in ("q", "k"):
                    rope_out[(t, h)] = mid.tile([96, S], BF16,
                                                name=f"r_{t}{h}",
                                                tag=f"r_{t}{h}")

            # ---- phase 1: projections + rope + v layout ----
            with tc.tile_pool(name="p3", bufs=1) as p3, \
                 tc.tile_pool(name="p3ps", bufs=8, space="PSUM") as p3ps:
                wq_sb = p3.tile([128, KC * 192], BF16, name="wq_sb")
                nc.sync.dma_start(out=wq_sb[:], in_=wqd.ap())
                wk_sb = p3.tile([128, KC * 192], BF16, name="wk_sb")
                nc.sync.dma_start(out=wk_sb[:], in_=wkd.ap())
                wv_sb = p3.tile([128, KC * 192], BF16, name="wv_sb")
                nc.sync.dma_start(out=wv_sb[:], in_=wvd.ap())
                xt = []
                for kc in range(KC):
                    t_ = p3.tile([128, S], BF16, name=f"xt{kc}", tag="xt",
                                 bufs=KC)
                    xg_rows(kc * 128, 128, t_)
                    xt.append(t_)

                def project(wsb, bsb, h, dest):
                    ps = [p3ps.tile([96, 512], F32, name=f"ps{n}", tag="proj",
                                    bufs=6)
                          for n in range(NQ)]
                    for kc in range(KC):
                        lhs = wsb[:, kc * 192 + h * 96: kc * 192 + (h + 1) * 96]
                        for n in range(NQ):
                            nc.tensor.matmul(
                                ps[n][:], lhs,
                                xt[kc][:, n * 512:(n + 1) * 512],
                                start=(kc == 0), stop=(kc == KC - 1))
                    for n in range(NQ):
                        nc.vector.tensor_scalar_add(
                            dest[:, n * 512:(n + 1) * 512], ps[n][:],
                            bsb[:, h:h + 1])

                for h in (0, 1):
                    for (t, wsb, bsb) in (("q", wq_sb, bq_sb),
                                          ("k", wk_sb, bk_sb)):
                        pre = p3.tile([96, S], BF16, name=f"pre_{t}{h}",
                                      tag="pre", bufs=2)
                        project(wsb, bsb, h, pre)
                        # rope: dst = pre*cosF + swap(pre)*sinF
                        sw = p3.tile([96, S], BF16, name=f"sw_{t}{h}",
                                     tag="sw", bufs=1)
                        nc.sync.dma_start(out=sw[0:32, :], in_=pre[0:32, :])
                        nc.sync.dma_start(out=sw[32:64, :], in_=pre[64:96, :])
                        nc.sync.dma_start(out=sw[64:96, :], in_=pre[32:64, :])
                        t1 = p3.tile([96, S], BF16, name=f"t1_{t}{h}",
                                     tag="t1", bufs=1)
                        nc.vector.tensor_tensor(t1[:], pre[:], cos_sb[:],
                                                OP.mult)
                        t2 = p3.tile([96, S], BF16, name=f"t2_{t}{h}",
                                     tag="t2", bufs=1)
                        nc.vector.tensor_tensor(t2[:], sw[:], sin_sb[:],
                                                OP.mult)
                        nc.vector.tensor_tensor(rope_out[(t, h)][:], t1[:],
                                                t2[:], OP.add)
                    vt_h = p3.tile([96, S], BF16, name=f"vt{h}", tag="vt",
                                   bufs=1)
                    project(wv_sb, bv_sb, h, vt_h)
                    for kc2 in range(NKC):
                        tr = p3ps.tile([128, 96], BF16,
                                       name=f"tr{h}_{kc2}", tag="tr",
                                       bufs=2)
                        nc.tensor.transpose(
                            tr[:], vt_h[:, kc2 * 128:(kc2 + 1) * 128],
                            ident[:])
                        nc.vector.tensor_copy(
                            v1[h][:, kc2 * 128: kc2 * 128 + 96], tr[:])
                    ones_ap = v1[h].rearrange("p (k c) -> p k c", c=128)
                    nc.gpsimd.memset(ones_ap[:, :, 96:97], 1.0)

            # ---- phase 2: attention (+ per-head gather as it finishes) ----
            with tc.tile_pool(name="p5", bufs=1) as p5, \
                 tc.tile_pool(name="p5ps", bufs=2, space="PSUM") as ps_s, \
                 tc.tile_pool(name="pvps", bufs=2, space="PSUM") as ps_pv:
                for qb in range(NQB):
                    for h in (0, 1):
                        qT = rope_out[("q", h)]
                        kT = rope_out[("k", h)]
                        q0 = qb * QB
                        pv0 = ps_pv.tile([97, 512], F32, name=f"pv0_{h}{qb}",
                                         tag="pv0", bufs=2)
                        pv1 = ps_pv.tile([97, 512], F32, name=f"pv1_{h}{qb}",
                                         tag="pv1", bufs=2)
                        for kc2 in range(NKC):
                            sps = ps_s.tile([128, 1024], F32,
                                            name=f"s_{h}{qb}_{kc2}", tag="s",
                                            bufs=2)
                            lhs_k = kT[:, kc2 * 128:(kc2 + 1) * 128]
                            nc.tensor.matmul(sps[:, 0:512], lhs_k,
                                             qT[:, q0: q0 + 512])
                            nc.tensor.matmul(sps[:, 512:1024], lhs_k,
                                             qT[:, q0 + 512: q0 + 1024])
                            pt = p5.tile([128, 1024], BF16,
                                         name=f"pt_{h}{qb}_{kc2}", tag="pt",
                                         bufs=3)
                            nc.scalar.activation(pt[:], sps[:], AF.Exp,
                                                 scale=SCALE)
                            lhs_v = v1[h][:, kc2 * 128: kc2 * 128 + 97]
                            nc.tensor.matmul(pv0[:], lhs_v, pt[:, 0:512],
                                             start=(kc2 == 0),
                                             stop=(kc2 == NKC - 1))
                            nc.tensor.matmul(pv1[:], lhs_v, pt[:, 512:1024],
                                             start=(kc2 == 0),
                                             stop=(kc2 == NKC - 1))
                        for j, pv in enumerate((pv0, pv1)):
                            den = p5.tile([97, 512], F32,
                                          name=f"den{h}{qb}{j}",
                                          tag="den", bufs=2)
                            nc.vector.tensor_copy(den[96:97, :], pv[96:97, :])
                            dn0 = p5.tile([1, 512], F32, name=f"dn0{h}{qb}{j}",
                                          tag="dn0", bufs=2)
                            nc.sync.dma_start(out=dn0[:], in_=den[96:97, :])
                            rec = p5.tile([1, 512], F32, name=f"rec{h}{qb}{j}",
                                          tag="rec", bufs=2)
                            nc.vector.reciprocal(rec[:], dn0[:])
                            rb = p5.tile([96, 512], F32, name=f"rb{h}{qb}{j}",
                                         tag="rb", bufs=2)
                            nc.gpsimd.partition_broadcast(rb[:], rec[:])
                            nc.vector.tensor_tensor(
                                attn[h][:, q0 + j * 512: q0 + (j + 1) * 512],
                                pv[0:96, :], rb[:], OP.mult)
                        # once both heads finished this column block, gather
                        # it while the next rounds compute
                        if h == 1:
                            agi, _ago = ag_c[qb]
                            for hh in (0, 1):
                                nc.sync.dma_start(
                                    out=agi[hh * 96:(hh + 1) * 96, :],
                                    in_=attn[hh][:, q0:q0 + QB])
                            nc.gpsimd.collective_compute(
                                "AllGather", mybir.AluOpType.bypass,
                                replica_groups=groups,
                                ins=[agi[:]], outs=[_ago[:]])

            # ---- phase 3: output projection (my 192 columns of o),
            #      int8 row-quantized download ----
            with tc.tile_pool(name="p6", bufs=1) as p6, \
                 tc.tile_pool(name="p6ps", bufs=4, space="PSUM") as p6ps:
                at = []
                for kc in range(KC):
                    t_ = p6.tile([128, S], BF16, name=f"at{kc}", tag="at",
                                 bufs=KC)
                    R, R1 = 128 * kc, 128 * (kc + 1)
                    while R < R1:
                        head = R // 96
                        take = min(R1 - R, 96 - R % 96)
                        for qb in range(NQB):
                            nc.sync.dma_start(
                                out=t_[R - 128 * kc: R - 128 * kc + take,
                                       qb * QB:(qb + 1) * QB],
                                in_=ag_c[qb][1][
                                    head // 2,
                                    (head % 2) * 96 + R % 96:
                                    (head % 2) * 96 + R % 96 + take, :])
                        R += take
                    at.append(t_)
                orow = {}
                for g in (0, 1):
                    orow[g] = p6.tile([96, S], F32, name=f"orow{g}",
                                      tag=f"orow{g}")
                for n in range(NQ):
                    for g in (0, 1):
                        po = p6ps.tile([96, 512], F32, name=f"po{n}_{g}",
                                       tag="po")
                        for kc in range(KC):
                            nc.tensor.matmul(
                                po[:],
                                wo_sb[:, kc * 192 + g * 96:
                                      kc * 192 + (g + 1) * 96],
                                at[kc][:, n * 512:(n + 1) * 512],
                                start=(kc == 0), stop=(kc == KC - 1))
                        if (2 * n + g) % 2:
                            nc.scalar.activation(
                                orow[g][:, n * 512:(n + 1) * 512], po[:],
                                AF.Copy)
                        else:
                            nc.vector.tensor_copy(
                                orow[g][:, n * 512:(n + 1) * 512], po[:])
                for g in (0, 1):
                    rowmax = p6.tile([96, 1], F32, name=f"rmax{g}",
                                     tag=f"rmax{g}")
                    nc.vector.tensor_reduce(
                        rowmax[:], orow[g][:], axis=mybir.AxisListType.X,
                        op=OP.max, apply_absolute_value=True)
                    rinv = p6.tile([96, 1], F32, name=f"rinv{g}",
                                   tag=f"rinv{g}")
                    nc.vector.reciprocal(rinv[:], rowmax[:])
                    qinv = p6.tile([96, 1], F32, name=f"qinv{g}",
                                   tag=f"qinv{g}")
                    nc.scalar.activation(qinv[:], rinv[:], AF.Copy,
                                         scale=127.0)
                    sc = p6.tile([96, 1], F32, name=f"sc{g}", tag=f"sc{g}")
                    nc.scalar.activation(sc[:], rowmax[:], AF.Copy,
                                         scale=1.0 / 127.0)
                    nc.sync.dma_start(out=oscd.ap()[g * 96:(g + 1) * 96, :],
                                      in_=sc[:])
                    q8 = p6.tile([96, S], I8, name=f"q8{g}", tag=f"q8{g}")
                    nc.vector.tensor_scalar_mul(q8[:], orow[g][:],
                                                qinv[:, 0:1])
                    nc.sync.dma_start(out=outd.ap()[g * 96:(g + 1) * 96, :],
                                      in_=q8[:])
            mid.release()
            const.release()

    nc.compile()
    return nc


def _get_nc():
    if "nc" not in _STATE:
        _STATE["nc"] = _build_nc()
    return _STATE["nc"]


def make_in_maps(x, wq, bq, wk, bk, wv, bv, wo, bo, freqs_cos, freqs_sin,
                 h, w):
    """Host-side shard prep: pure indexing/casting, returns per-core in_maps."""
    assert int(h) == GH and int(w) == GW
    x = np.asarray(x, np.float32)
    wq = np.asarray(wq, np.float32)
    wk = np.asarray(wk, np.float32)
    wv = np.asarray(wv, np.float32)
    wo = np.asarray(wo, np.float32)
    bq = np.asarray(bq, np.float32)
    bk = np.asarray(bk, np.float32)
    bv = np.asarray(bv, np.float32)
    fc = np.asarray(freqs_cos, np.float32)
    fs = np.asarray(freqs_sin, np.float32)

    perm = np.asarray(_PERM)

    # rope tables in the permuted row basis
    tpos = np.arange(S)
    gh = tpos // GW
    gw = tpos % GW
    c32 = np.empty((32, S), np.float32)
    s32 = np.empty((32, S), np.float32)
    c32[0:16] = fc[gh, 16:32].T
    c32[16:32] = fc[gw, 32:48].T
    s32[0:16] = fs[gh, 16:32].T
    s32[16:32] = fs[gw, 32:48].T
    cosF = np.ones((96, S), np.float32)
    cosF[32:64] = c32
    cosF[64:96] = c32
    sinF = np.zeros((96, S), np.float32)
    sinF[32:64] = -s32
    sinF[64:96] = s32
    cosF = cosF.astype(BF16N)
    sinF = sinF.astype(BF16N)

    xT = np.ascontiguousarray(x[0].astype(BF16N).T)   # [DIM, S]

    def tile_w(wc):
        # [1536, 192] -> [128, KC*192] with col block kc = rows kc*128..+128
        return np.ascontiguousarray(
            wc.reshape(KC, 128, 192).transpose(1, 0, 2).reshape(128, KC * 192)
        ).astype(BF16N)

    in_maps = []
    for c in range(NCORES):
        h0, h1 = 2 * c, 2 * c + 1
        sl = slice(c * SS, (c + 1) * SS)
        xcs = np.empty((XR, SS), BF16N)
        xcs[0:DIM] = xT[:, sl]
        xcs[DIM:DIM + 96] = cosF[:, sl]
        xcs[DIM + 96:DIM + 192] = sinF[:, sl]
        qk_cols = np.concatenate([h0 * HD + perm, h1 * HD + perm])
        v_cols = np.arange(h0 * HD, (h1 + 1) * HD)
        bq_c = np.stack([bq[h0 * HD + perm], bq[h1 * HD + perm]], axis=1)
        bk_c = np.stack([bk[h0 * HD + perm], bk[h1 * HD + perm]], axis=1)
        bv_c = np.stack([bv[v_cols[:HD]], bv[v_cols[HD:]]], axis=1)
        in_maps.append({
            "xcs_s": xcs,
            "wq_t": tile_w(wq[:, qk_cols]),
            "wk_t": tile_w(wk[:, qk_cols]),
            "wv_t": tile_w(wv[:, v_cols]),
            "wo_t": tile_w(wo[:, 2 * HD * c: 2 * HD * (c + 1)]),
            "bq_t": np.ascontiguousarray(bq_c, dtype=np.float32),
            "bk_t": np.ascontiguousarray(bk_c, dtype=np.float32),
            "bv_t": np.ascontiguousarray(bv_c, dtype=np.float32),
        })
    return in_maps


def assemble_output(parts, bo):
    """parts: list of 8 (o8 [192, S] int8, osc [192, 1] f32) tuples."""
    o8 = np.concatenate([np.asarray(p[0]) for p in parts], axis=0)
    osc = np.concatenate([np.asarray(p[1]) for p in parts], axis=0)
    oT = o8.astype(np.float32) * osc                     # [DIM, S]
    out = oT.T + np.asarray(bo, np.float32)[None, :]
    return out[None]


def kernel(x, wq, bq, wk, bk, wv, bv, wo, bo, freqs_cos, freqs_sin, h, w):
    from concourse.bass_utils import run_bass_kernel_spmd

    nc = _get_nc()
    in_maps = make_in_maps(x, wq, bq, wk, bk, wv, bv, wo, bo,
                           freqs_cos, freqs_sin, h, w)
    res = run_bass_kernel_spmd(nc, in_maps, core_ids=list(range(NCORES)))
    parts = [(res.results[c]["o8"], res.results[c]["osc"])
             for c in range(NCORES)]
    return assemble_output(parts, bo)
